# revision 10
# baseline (speedup 1.0000x reference)
"""Trainium2 Bass kernel for BidirectionalCrossAttentionGate.

Data-parallel over batch B=8 across 8 NeuronCores (1 batch element/core).
Per core, one SBUF-resident pipeline; attention path runs fp8e4 DoubleRow
matmuls (2x PE throughput), gating + final projection run bf16/fp32:

  P0: load X, C [2048,768] fp32, PE-transpose -> Xt, Ct fp8 [768,2048]
  per direction:
    P1: Qt, Kt fp8 [768,2048] and V fp8 [2048,768] via fp8-DR matmuls,
        biases folded in (per-partition ACT bias / bf16 ones-row matmul)
    A3: per k-tile: scores PSUM = DR-matmuls(Kt-slice, Qt);
        exp(s/sqrt(D)) -> bf16 tmp + row-sums Z[k]; wt_fp8 = tmp * (2048/Z[k])
    A4: attn*2048 accumulated over k via DR-matmuls(wt, V);
        a = sigmoid(tanh(psum/2048)) -> bf16 -> DRAM scratch
  G:  stream X, C fp32; out1 = c + a2*(x-c); out2 = x - a1*(x-c);
      PE-transpose -> fusedT bf16; Y = fusedT^T @ Wf + bf (bf16 matmul)

softmax is over the QUERY axis (jnp softmax dim=1) = the free axis of the
[k, q] score tiles; row sums per k come from the Exp activation's accum_out.
Max-subtraction is skipped: |s/sqrt(D)| stays < ~2 for randn-scale inputs.
"""
import numpy as np
import ml_dtypes
from contextlib import ExitStack

import concourse.bass as bass
import concourse.tile as tile
from concourse import bacc, mybir
from concourse.bass_utils import run_bass_kernel_spmd
from concourse.masks import make_identity

B, S, D = 8, 2048, 768
P = 128
NST = S // P          # 16 s-tiles
NDT = D // P          # 6 d-tiles
AF = mybir.ActivationFunctionType
ALU = mybir.AluOpType
F32 = mybir.dt.float32
BF16 = mybir.dt.bfloat16
FP8 = mybir.dt.float8e4
DR = mybir.MatmulPerfMode.DoubleRow
SCALE = 1.0 / float(np.sqrt(float(D)))
ZS = 2048.0           # wt pre-scale so fp8 weights stay O(1)

USE_FP8 = True
ADT = FP8 if USE_FP8 else BF16
ADT_NP = ml_dtypes.float8_e4m3 if USE_FP8 else ml_dtypes.bfloat16

_NC_CACHE = {}


def _proj_mms(nc, ps_ap, lhs_tile, l_sl, rhs_tile, r_sl, close=True):
    """Contraction over NDT d-tiles: DR pairs for fp8, singles for bf16."""
    if USE_FP8:
        n = NDT // 2
        for t in range(n):
            nc.tensor.matmul(ps_ap, lhs_tile[:, 2 * t:2 * t + 2, l_sl],
                             rhs_tile[:, 2 * t:2 * t + 2, r_sl],
                             start=(t == 0), stop=(close and t == n - 1), perf_mode=DR)
    else:
        for t in range(NDT):
            nc.tensor.matmul(ps_ap, lhs_tile[:, t, l_sl], rhs_tile[:, t, r_sl],
                             start=(t == 0), stop=(close and t == NDT - 1))


def _build_nc(rep=1):
    nc = bacc.Bacc("TRN2", target_bir_lowering=False, debug=False, num_devices=8)

    x_d = nc.declare_dram_parameter("x", [S, D], F32, isOutput=False)
    c_d = nc.declare_dram_parameter("c", [S, D], F32, isOutput=False)
    w_d = {}
    for nm in ("wq1", "wk1", "wv1", "wq2", "wk2", "wv2"):
        w_d[nm] = nc.declare_dram_parameter(nm, [D, D], ADT, isOutput=False)
    wf_d = nc.declare_dram_parameter("wf", [2 * D, D], BF16, isOutput=False)
    b_d = {}
    for nm in ("bq1", "bk1", "bv1", "bq2", "bk2", "bv2", "bfin"):
        b_d[nm] = nc.declare_dram_parameter(nm, [D], F32, isOutput=False)
    y_d = nc.declare_dram_parameter("y", [S, D], F32, isOutput=True)

    a1_d = nc.dram_tensor("a1scr", [S, D], BF16)
    a2_d = nc.dram_tensor("a2scr", [S, D], BF16)

    with tile.TileContext(nc) as tc, ExitStack() as octx:
        pmisc = octx.enter_context(tc.tile_pool(name="pmisc", bufs=1))
        pps = octx.enter_context(tc.tile_pool(name="pps", bufs=4, space="PSUM"))
        psc = octx.enter_context(tc.tile_pool(name="psc", bufs=2, space="PSUM"))

        ident = pmisc.tile([P, P], F32)
        make_identity(nc, ident[:])
        ones_row = pmisc.tile([1, P], BF16)
        nc.gpsimd.memset(ones_row[:], 1.0)
        bias_pp = {}
        for nm in ("bq1", "bk1", "bq2", "bk2"):
            t = pmisc.tile([P, NDT], F32, tag=f"bp_{nm}")
            nc.sync.dma_start(t[:], b_d[nm][:].rearrange("(j p) -> p j", p=P))
            bias_pp[nm] = t
        bias_row = {}
        for nm in ("bv1", "bv2", "bfin"):
            t = pmisc.tile([1, D], BF16, tag=f"br_{nm}")
            nc.gpsimd.dma_start(t[:], b_d[nm][:].rearrange("(a d) -> a d", a=1))
            bias_row[nm] = t

        for _r in range(rep):
            with tc.tile_pool(name=f"pxt{_r}", bufs=1) as pxt:
                xt = pxt.tile([P, NDT, S], ADT, tag="xt")
                ct = pxt.tile([P, NDT, S], ADT, tag="ct")

                # ---- P0: load + PE-transpose X, C ----
                with tc.tile_pool(name=f"pstage{_r}", bufs=6) as pstage:
                    for src_d, dst in ((x_d, xt), (c_d, ct)):
                        for blk in range(4):
                            stgs = []
                            for i in range(4):
                                st = blk * 4 + i
                                g = pstage.tile([P, D], F32, tag="stg")
                                nc.sync.dma_start(g[:], src_d[st * P:(st + 1) * P, :])
                                stgs.append(g)
                            for j in range(NDT):
                                tp = pps.tile([P, 512], F32, tag="ps")
                                for i in range(4):
                                    nc.tensor.transpose(
                                        tp[:, i * P:(i + 1) * P],
                                        stgs[i][:, j * P:(j + 1) * P], ident[:])
                                nc.scalar.activation(
                                    dst[:, j, blk * 512:(blk + 1) * 512], tp[:],
                                    AF.Identity)

                with (
                    tc.tile_pool(name=f"pw{_r}", bufs=2) as pw,
                    tc.tile_pool(name=f"pbig{_r}", bufs=2) as pbig,
                    tc.tile_pool(name=f"pwt{_r}", bufs=1) as pwt,
                    tc.tile_pool(name=f"psm{_r}", bufs=4) as psm,
                    tc.tile_pool(name=f"pat{_r}", bufs=3) as pat,
                ):
                    for wq_nm, wk_nm, wv_nm, q_src, k_src, v_src, a_dst in (
                        ("wq1", "wk1", "wv1", xt, ct, ct, a1_d),
                        ("wq2", "wk2", "wv2", ct, xt, xt, a2_d),
                    ):
                        # ---- P1: Qt, Kt (transposed), V (natural) ----
                        proj = []
                        for w_nm, src in ((wq_nm, q_src), (wk_nm, k_src)):
                            w_sb = pw.tile([P, NDT, D], ADT, tag="w")
                            nc.sync.dma_start(
                                w_sb[:], w_d[w_nm][:].rearrange("(j p) n -> p j n", p=P))
                            out_t = pbig.tile([P, NDT, S], ADT, tag="b24")
                            bcol = bias_pp[w_nm.replace("w", "b")]
                            for j in range(NDT):
                                for ch in range(4):
                                    ps = pps.tile([P, 512], F32, tag="ps")
                                    _proj_mms(nc, ps[:], w_sb, slice(j * P, (j + 1) * P),
                                              src, slice(ch * 512, (ch + 1) * 512))
                                    nc.scalar.activation(
                                        out_t[:, j, ch * 512:(ch + 1) * 512], ps[:],
                                        AF.Identity, bias=bcol[:, j:j + 1])
                            proj.append(out_t)
                        qt, kt_t = proj

                        # ---- A3: scores -> exp -> normalized fp8 weights ----
                        wt = pwt.tile([P, NST, S], ADT, tag="wt")
                        for kt in range(NST):
                            wtmp = psm.tile([P, S], BF16, tag="wtmp")
                            zsum = psm.tile([P, 2], F32, tag="zsum")
                            for c in range(2):
                                sc = psc.tile([P, 1024], F32, tag="sc")
                                for h in range(2):
                                    _proj_mms(nc, sc[:, h * 512:(h + 1) * 512],
                                              kt_t, slice(kt * P, (kt + 1) * P),
                                              qt, slice((2 * c + h) * 512, (2 * c + h + 1) * 512))
                                nc.scalar.activation(
                                    wtmp[:, c * 1024:(c + 1) * 1024], sc[:],
                                    AF.Exp, scale=SCALE, accum_out=zsum[:, c:c + 1])
                            ztot = psm.tile([P, 1], F32, tag="ztot")
                            nc.vector.tensor_tensor(ztot[:], zsum[:, 0:1], zsum[:, 1:2], ALU.add)
                            zofs = psm.tile([P, 1], F32, tag="zofs")
                            nc.vector.tensor_scalar_mul(zofs[:], ztot[:], 1.0 / ZS)
                            invz = psm.tile([P, 1], F32, tag="invz")
                            nc.vector.reciprocal(invz[:], zofs[:])
                            nc.vector.tensor_scalar_mul(wt[:, kt, :], wtmp[:], invz[:])

                        # ---- V projection (natural) ----
                        w_sb = pw.tile([P, NDT, D], ADT, tag="w")
                        nc.sync.dma_start(
                            w_sb[:], w_d[wv_nm][:].rearrange("(j p) n -> p j n", p=P))
                        v = pbig.tile([P, NST, D], ADT, tag="b24")
                        bv_row = bias_row[wv_nm.replace("w", "b")]
                        for st in range(NST):
                            for c0, cn in ((0, 512), (512, 256)):
                                ps = pps.tile([P, 512], F32, tag="ps")
                                _proj_mms(nc, ps[:, :cn], v_src, slice(st * P, (st + 1) * P),
                                          w_sb, slice(c0, c0 + cn), close=False)
                                nc.tensor.matmul(
                                    ps[:, :cn], ones_row[:, :], bv_row[:, c0:c0 + cn],
                                    start=False, stop=True)
                                nc.scalar.activation(v[:, st, c0:c0 + cn], ps[:, :cn],
                                                     AF.Identity)

                        # ---- A4: attn (x ZS) -> gates -> DRAM scratch ----
                        for qt_i in range(NST):
                            th = psm.tile([P, D], F32, tag="th")
                            at = pat.tile([P, D], BF16, tag="at")
                            for c0, cn in ((0, 384), (384, 384)):
                                ps = pps.tile([P, 512], F32, tag="ps")
                                if USE_FP8:
                                    for t in range(NST // 2):
                                        nc.tensor.matmul(
                                            ps[:, :cn],
                                            wt[:, 2 * t:2 * t + 2, qt_i * P:(qt_i + 1) * P],
                                            v[:, 2 * t:2 * t + 2, c0:c0 + cn],
                                            start=(t == 0), stop=(t == NST // 2 - 1),
                                            perf_mode=DR)
                                else:
                                    for t in range(NST):
                                        nc.tensor.matmul(
                                            ps[:, :cn], wt[:, t, qt_i * P:(qt_i + 1) * P],
                                            v[:, t, c0:c0 + cn],
                                            start=(t == 0), stop=(t == NST - 1))
                                nc.scalar.activation(th[:, c0:c0 + cn], ps[:, :cn],
                                                     AF.Tanh, scale=1.0 / ZS)
                                nc.scalar.activation(at[:, c0:c0 + cn], th[:, c0:c0 + cn],
                                                     AF.Sigmoid)
                            nc.sync.dma_start(a_dst[qt_i * P:(qt_i + 1) * P, :], at[:])

            # ---- G: gating + final projection ----
            with (
                tc.tile_pool(name=f"pg{_r}", bufs=4) as pg,
                tc.tile_pool(name=f"pgw{_r}", bufs=1) as pgw,
                tc.tile_pool(name=f"pgo{_r}", bufs=3) as pgo,
            ):
                wf_sb = pgw.tile([P, 2 * NDT, D], BF16, tag="wf")
                nc.sync.dma_start(wf_sb[:], wf_d[:].rearrange("(j p) n -> p j n", p=P))
                bf_row = bias_row["bfin"]

                for st in range(NST):
                    xg = pg.tile([P, D], F32, tag="xg")
                    nc.sync.dma_start(xg[:], x_d[st * P:(st + 1) * P, :])
                    cg = pg.tile([P, D], F32, tag="cg")
                    nc.sync.dma_start(cg[:], c_d[st * P:(st + 1) * P, :])
                    a1t = pg.tile([P, D], BF16, tag="a1g")
                    nc.sync.dma_start(a1t[:], a1_d[st * P:(st + 1) * P, :])
                    a2t = pg.tile([P, D], BF16, tag="a2g")
                    nc.sync.dma_start(a2t[:], a2_d[st * P:(st + 1) * P, :])

                    dd = pgo.tile([P, D], F32, tag="dd")
                    nc.vector.tensor_tensor(dd[:], xg[:], cg[:], ALU.subtract)
                    m2 = pgo.tile([P, D], F32, tag="m2")
                    nc.vector.tensor_tensor(m2[:], a2t[:], dd[:], ALU.mult)
                    out1 = pgo.tile([P, D], F32, tag="o1")
                    nc.vector.tensor_tensor(out1[:], cg[:], m2[:], ALU.add)
                    m1 = pgo.tile([P, D], F32, tag="m1")
                    nc.vector.tensor_tensor(m1[:], a1t[:], dd[:], ALU.mult)
                    out2 = pgo.tile([P, D], F32, tag="o2")
                    nc.vector.tensor_tensor(out2[:], xg[:], m1[:], ALU.subtract)

                    ft = pgo.tile([P, 2 * NDT * P], BF16, tag="ft")
                    for g in range(3):
                        tp = pps.tile([P, 512], F32, tag="ps")
                        for i in range(4):
                            jj = g * 4 + i
                            src = out1 if jj < NDT else out2
                            jloc = jj if jj < NDT else jj - NDT
                            nc.tensor.transpose(
                                tp[:, i * P:(i + 1) * P],
                                src[:, jloc * P:(jloc + 1) * P], ident[:])
                        nc.scalar.activation(ft[:, g * 512:(g + 1) * 512], tp[:], AF.Identity)

                    yt = pgo.tile([P, D], F32, tag="yt")
                    for c0, cn in ((0, 384), (384, 384)):
                        ps = pps.tile([P, 512], F32, tag="ps")
                        for j in range(2 * NDT):
                            nc.tensor.matmul(
                                ps[:, :cn], ft[:, j * P:(j + 1) * P], wf_sb[:, j, c0:c0 + cn],
                                start=(j == 0), stop=False)
                        nc.tensor.matmul(
                            ps[:, :cn], ones_row[:, :], bf_row[:, c0:c0 + cn],
                            start=False, stop=True)
                        nc.scalar.activation(yt[:, c0:c0 + cn], ps[:, :cn], AF.Identity)
                    nc.sync.dma_start(y_d[st * P:(st + 1) * P, :], yt[:])

    nc.compile()
    return nc


def _get_nc():
    if "nc" not in _NC_CACHE:
        _NC_CACHE["nc"] = _build_nc()
    return _NC_CACHE["nc"]


def kernel(**inputs):
    nc = _get_nc()
    bf16 = ml_dtypes.bfloat16
    shared = {
        "wq1": inputs["Wq1"].astype(ADT_NP), "wk1": inputs["Wk1"].astype(ADT_NP),
        "wv1": inputs["Wv1"].astype(ADT_NP), "wq2": inputs["Wq2"].astype(ADT_NP),
        "wk2": inputs["Wk2"].astype(ADT_NP), "wv2": inputs["Wv2"].astype(ADT_NP),
        "wf": inputs["Wf"].astype(bf16),
        "bq1": inputs["bq1"], "bk1": inputs["bk1"], "bv1": inputs["bv1"],
        "bq2": inputs["bq2"], "bk2": inputs["bk2"], "bv2": inputs["bv2"],
        "bfin": inputs["bf"],
    }
    in_maps = []
    for b in range(B):
        m = dict(shared)
        m["x"] = np.ascontiguousarray(inputs["self_x"][b])
        m["c"] = np.ascontiguousarray(inputs["conv_x"][b])
        in_maps.append(m)
    res = run_bass_kernel_spmd(nc, in_maps, list(range(B)))
    return np.stack([res.results[b]["y"] for b in range(B)], axis=0)
